# revision 43
# baseline (speedup 1.0000x reference)
"""Trainium2 Bass kernel for causal self-attention with log1p-distance decay bias.

Problem (hardcoded shapes): x [4, 2048, 1024], w_attn [1024, 3072],
w_proj [1024, 1024], decay_raw [16]; 16 heads, head dim 64.

Sharding over 8 cores: core c -> (batch b = c//2, head-group g = c%2).
Each core computes its batch's qkv for its 8 heads, attention in
"S-transposed" layout (k on partitions, q on free dim), then a partial
projection out_p = y_g @ w_proj[rows of g]  [2048, 1024]. Host sums the
two partials per batch.

The causal + decay bias  exp(-log1p(softplus(decay)*log1p(q-k)))  is a
Toeplitz function of d = q - k, materialized per head as one [128, 2048]
"strip" whose column c at partition p holds the value for d = c - p; the
tile for k-chunk kc / q-window [q0, q0+nq) is the contiguous strip slice
[q0-128*kc, q0-128*kc+nq). d < 0 (future) entries are zeroed, which also
implements the causal mask (P = exp(s) * strip = 0 there).

Softmax denominators come free from a ones-column appended to v (no
running max is needed: scores are O(+-6) so exp never overflows).

All matmuls run in float32r (~1.3e-4 rel err, 4x faster than fp32).
"""

import numpy as np

import concourse.bass as bass
import concourse.mybir as mybir
import concourse.tile as tile
from concourse import bacc
from concourse.bass_utils import run_bass_kernel_spmd

B, T, C, H = 4, 2048, 1024, 16
HG = 8  # heads per core
D = 64
N_CORES = 8
F32 = mybir.dt.float32
F32R = mybir.dt.float32r
AF = mybir.ActivationFunctionType
ALU = mybir.AluOpType

_CACHE = {}


def _body(nc, tc, io, ctx):
    xT, wqk, wv, wp, dec, Lc, A0, ones_c, outp = io

    singles = ctx.enter_context(tc.tile_pool(name="singles", bufs=1))

    # ---------------- phase 1: qkv ----------------
    # qT/kT: [128 rows = 2 heads x 64 dims, 2048 pos] per col-chunk cc.
    qkt_pool = ctx.enter_context(tc.tile_pool(name="qkt", bufs=1))
    qT = [qkt_pool.tile([128, T], F32R, tag=f"qT{t}", name=f"qT{t}") for t in range(4)]
    kT = [qkt_pool.tile([128, T], F32R, tag=f"kT{t}", name=f"kT{t}") for t in range(4)]
    v_aug = qkt_pool.tile([128, 16, HG, D + 1], F32R, tag="vaug")
    # ones column of v_aug (denominator trick), from DRAM so the fp32r
    # producer chain is DMA-only.
    nc.sync.dma_start(
        out=v_aug[:, :, :, D : D + 1],
        in_=ones_c.rearrange("p (a b) -> p a b", a=16).unsqueeze(-1),
    )

    with tc.tile_pool(name="wqk", bufs=1) as wqk_pool, \
         tc.tile_pool(name="xq", bufs=2) as xq_pool, \
         tc.tile_pool(name="psA", bufs=4, space="PSUM") as psA:
        # first x tile before the weights: the first matmul needs only
        # xq[0] + the t=0 weight slice, so don't queue 6MB of weights first
        xq0 = xq_pool.tile([128, 8, 512], F32R, name="xq", tag="xq")
        nc.sync.dma_start(
            out=xq0[:], in_=xT[:, 0:512].rearrange("(c p) n -> p c n", p=128))
        wqk_sb = wqk_pool.tile([128, 8, 2 * HG * D], F32R)
        # split per column-chunk so the first matmul can start after ~512KB
        for t in range(8):
            nc.sync.dma_start(
                out=wqk_sb[:, :, t * 128 : (t + 1) * 128],
                in_=wqk[:, t * 128 : (t + 1) * 128].rearrange(
                    "(c p) n -> p c n", p=128),
            )
        wv_sb = wqk_pool.tile([128, 8, HG * D], F32R)
        nc.sync.dma_start(out=wv_sb[:], in_=wv.rearrange("(c p) n -> p c n", p=128))
        for pq in range(4):
            if pq == 0:
                xq = xq0
            else:
                xq = xq_pool.tile([128, 8, 512], F32R, name="xq", tag="xq")
                nc.sync.dma_start(
                    out=xq[:],
                    in_=xT[:, pq * 512 : (pq + 1) * 512].rearrange(
                        "(c p) n -> p c n", p=128
                    ),
                )
            for t in range(8):
                ps = psA.tile([128, 512], F32, tag="psA")
                for c in range(8):
                    nc.tensor.matmul(
                        out=ps[:],
                        lhsT=wqk_sb[:, c, t * 128 : (t + 1) * 128],
                        rhs=xq[:, c, :],
                        start=(c == 0),
                        stop=(c == 7),
                    )
                # (1/sqrt(D) is pre-folded into wq on the host)
                dst = qT[t] if t < 4 else kT[t - 4]
                sl = dst[:, pq * 512 : (pq + 1) * 512]
                if t < 4:
                    nc.vector.tensor_copy(out=sl, in_=ps[:])
                else:
                    nc.scalar.activation(out=sl, in_=ps[:], func=AF.Copy)
            # v for the 4 pos-128-chunks inside this pq
            for i in range(4):
                p16 = pq * 4 + i
                psv = psA.tile([128, 512], F32, tag="psA")
                for c in range(8):
                    nc.tensor.matmul(
                        out=psv[:],
                        lhsT=xq[:, c, i * 128 : (i + 1) * 128],
                        rhs=wv_sb[:, c, :],
                        start=(c == 0),
                        stop=(c == 7),
                    )
                nc.vector.tensor_copy(
                    out=v_aug[:, p16, :, 0:D],
                    in_=psv.rearrange("p (h d) -> p h d", h=HG),
                )

    # ---------------- phase 2: attention ----------------
    # (constants loaded here, not at kernel start, so the phase-1 weight/x
    # DMAs own the DMA engines during startup)
    L_sb = singles.tile([128, T], F32)
    nc.sync.dma_start(out=L_sb[:], in_=Lc[:])
    A0_sb = singles.tile([128, 128], F32)
    nc.sync.dma_start(out=A0_sb[:], in_=A0[:])
    dec_b = singles.tile([128, HG], F32)
    nc.sync.dma_start(out=dec_b[:], in_=dec.to_broadcast([128, HG]))
    # softplus(x) = ln(exp(x) + 1) -- Softplus has no ACT table on gen3
    c_all = singles.tile([128, HG], F32)
    nc.scalar.activation(out=c_all[:], in_=dec_b[:], func=AF.Exp)
    nc.scalar.activation(out=c_all[:], in_=c_all[:], func=AF.Ln, bias=1.0)

    ypool = ctx.enter_context(tc.tile_pool(name="ypool", bufs=1))
    y = [ypool.tile([128, T], F32R, tag=f"y{t}", name=f"y{t}") for t in range(4)]

    with tc.tile_pool(name="strip", bufs=2) as strip_pool, \
         tc.tile_pool(name="pr", bufs=3) as pr_pool, \
         tc.tile_pool(name="rb", bufs=2) as rb_pool, \
         tc.tile_pool(name="yh", bufs=1) as yh_pool, \
         tc.tile_pool(name="sm", bufs=2) as sm_pool, \
         tc.tile_pool(name="wpp", bufs=1) as wp_pool, \
         tc.tile_pool(name="dsc", bufs=3, space="DRAM") as dsc_pool, \
         tc.tile_pool(name="psS", bufs=2, space="PSUM") as psS, \
         tc.tile_pool(name="psY", bufs=2, space="PSUM") as psY:
        wp_sb = wp_pool.tile([128, 4, C], F32R)
        nc.sync.dma_start(out=wp_sb[:], in_=wp.rearrange("(c p) n -> p c n", p=128))

        # projection chunks reuse the psS pool's PSUM slots (same [128,1024]
        # f32 shape) and pr's SBUF slots for eviction, so they can interleave
        # with the qh=1 attention pass instead of waiting for its pools.
        def proj_chunk(p16):
            pso = psS.tile([128, C], F32, tag="psS", name="pso")
            for cc2 in range(4):
                for nb in range(2):
                    nc.tensor.matmul(
                        out=pso[:, nb * 512 : (nb + 1) * 512],
                        lhsT=y[cc2][:, p16 * 128 : (p16 + 1) * 128],
                        rhs=wp_sb[:, cc2, nb * 512 : (nb + 1) * 512],
                        start=(cc2 == 0), stop=(cc2 == 3),
                    )
            oe = pr_pool.tile([128, C], F32, tag="pr", name="oe")
            if p16 % 2 == 0:
                nc.scalar.activation(out=oe[:], in_=pso[:], func=AF.Copy)
            else:
                nc.vector.tensor_copy(out=oe[:], in_=pso[:])
            nc.sync.dma_start(out=outp[p16 * 128 : (p16 + 1) * 128, :], in_=oe[:])

        for qh in range(2):
          for cc in range(4):
            for hl in range(2):
                h = 2 * cc + hl
                rows = slice(64 * hl, 64 * hl + 64)
                # strip[p, c] = exp(-log1p(c_h*L)) = 1/(1 + c_h*L), computed
                # on DVE (~18-bit recip) to keep the ACT engine free for exp
                strip = strip_pool.tile([128, T], F32, tag="strip")
                nc.gpsimd.tensor_scalar(
                    out=strip[:], in0=L_sb[:],
                    scalar1=c_all[:, h : h + 1], scalar2=1.0,
                    op0=ALU.mult, op1=ALU.add,
                )
                nc.vector.reciprocal_approx_fast(out=strip[:], in_=strip[:])
                # zero the d<0 (anti-causal) triangle, only in the first tile
                nc.gpsimd.tensor_tensor(
                    out=strip[:, 0:128], in0=strip[:, 0:128], in1=A0_sb[:],
                    op=ALU.mult,
                )
                for qh in range(2):
                    psy = psY.tile([65, 1024], F32, tag="psY")
                    kcs = [kc for kc in range(16) if 128 * kc < (qh + 1) * 1024]
                    for kc in kcs:
                        q0 = max(qh * 1024, 128 * kc)
                        nq = (qh + 1) * 1024 - q0
                        lo0 = q0 - qh * 1024  # local col in psy
                        sc0 = q0 - 128 * kc   # strip col
                        ps_s = psS.tile([128, 1024], F32, tag="psS")
                        for b0 in range(0, nq, 512):
                            w = min(512, nq - b0)
                            nc.tensor.matmul(
                                out=ps_s[:, b0 : b0 + w],
                                lhsT=kT[cc][rows, kc * 128 : (kc + 1) * 128],
                                rhs=qT[cc][rows, q0 + b0 : q0 + b0 + w],
                                start=True, stop=True,
                            )
                        # P = exp(s) * strip: exp PSUM->SBUF, then multiply in
                        # place (frees the PSUM tile as early as possible);
                        # spread the multiplies over DVE and GpSimd.
                        pr = pr_pool.tile([128, 1024], F32R, tag="pr")
                        nc.scalar.activation(out=pr[:, 0:nq], in_=ps_s[:, 0:nq],
                                             func=AF.Exp)
                        tt_eng = nc.gpsimd if kc % 5 in (1, 3) else nc.vector
                        tt_eng.tensor_tensor(
                            out=pr[:, 0:nq], in0=pr[:, 0:nq],
                            in1=strip[:, sc0 : sc0 + nq], op=ALU.mult,
                        )
                        # y_aug^T += v_aug[kc]^T @ P   (65 = 64 dims + denom)
                        # stop must land on the last matmul touching each
                        # 512-col PSUM zero region separately.
                        last_touch = {0: 3, 512: 7} if qh == 0 else {0: 11, 512: 15}
                        for b0 in range(0, 1024, 512):
                            lo = max(lo0, b0)
                            hi = min(lo0 + nq, b0 + 512)
                            if lo >= hi:
                                continue
                            nc.tensor.matmul(
                                out=psy[:, lo:hi],
                                lhsT=v_aug[:, kc, h, :],
                                rhs=pr[:, lo - lo0 : hi - lo0],
                                start=(kc == 0), stop=(kc == last_touch[b0]),
                            )
                    # normalize: y = y_aug[0:64] * (1 / denom)
                    # engines cannot shift partitions, and the custom-DVE
                    # recip only works at base partition 0 -> evict the denom
                    # row at base 64, DMA-broadcast via DRAM, recip at base 0.
                    rrow = sm_pool.tile([65, 1024], F32, tag="rrow")
                    nc.scalar.activation(out=rrow[64:65, :], in_=psy[64:65, :],
                                         func=AF.Copy)
                    dsc = dsc_pool.tile([1, 1024], F32, tag="dsc")
                    nc.sync.dma_start(out=dsc[:], in_=rrow[64:65, :])
                    rb = rb_pool.tile([64, 1024], F32, tag="rb")
                    nc.sync.dma_start(out=rb[:], in_=dsc.to_broadcast([64, 1024]))
                    nc.vector.reciprocal_approx_fast(out=rb[:], in_=rb[:])
                    if hl == 0:
                        nc.vector.tensor_tensor(
                            out=y[cc][0:64, qh * 1024 : (qh + 1) * 1024],
                            in0=psy[0:64, :], in1=rb[:], op=ALU.mult,
                        )
                    else:
                        yh = yh_pool.tile([64, 1024], F32R, tag="yh")
                        nc.vector.tensor_tensor(
                            out=yh[:], in0=psy[0:64, :], in1=rb[:], op=ALU.mult,
                        )
                        nc.sync.dma_start(
                            out=y[cc][64:128, qh * 1024 : (qh + 1) * 1024],
                            in_=yh[:],
                        )

    # ---------------- phase 3: projection ----------------
    with tc.tile_pool(name="oe", bufs=3) as oe_pool, \
         tc.tile_pool(name="wpp", bufs=1) as wp_pool, \
         tc.tile_pool(name="psO", bufs=3, space="PSUM") as psO:
        wp_sb = wp_pool.tile([128, 4, C], F32R)
        nc.sync.dma_start(out=wp_sb[:], in_=wp.rearrange("(c p) n -> p c n", p=128))
        for p16 in range(16):
            pso = psO.tile([128, C], F32, tag="psO")
            for cc in range(4):
                for nb in range(2):
                    nc.tensor.matmul(
                        out=pso[:, nb * 512 : (nb + 1) * 512],
                        lhsT=y[cc][:, p16 * 128 : (p16 + 1) * 128],
                        rhs=wp_sb[:, cc, nb * 512 : (nb + 1) * 512],
                        start=(cc == 0), stop=(cc == 3),
                    )
            oe = oe_pool.tile([128, C], F32, tag="oe")
            if p16 % 2 == 0:
                nc.scalar.activation(out=oe[:], in_=pso[:], func=AF.Copy)
            else:
                nc.vector.tensor_copy(out=oe[:], in_=pso[:])
            nc.sync.dma_start(out=outp[p16 * 128 : (p16 + 1) * 128, :], in_=oe[:])


def _build(reps=1):
    key = ("nc", reps)
    if key in _CACHE:
        return _CACHE[key]
    from contextlib import ExitStack

    nc = bacc.Bacc(None)
    xT = nc.dram_tensor("xT", [C, T], F32R, kind="ExternalInput")
    wqk = nc.dram_tensor("wqk", [C, 2 * HG * D], F32R, kind="ExternalInput")
    wv = nc.dram_tensor("wv", [C, HG * D], F32R, kind="ExternalInput")
    wp = nc.dram_tensor("wp", [HG * D, C], F32R, kind="ExternalInput")
    dec = nc.dram_tensor("dec", [1, HG], F32, kind="ExternalInput")
    Lc = nc.dram_tensor("Lc", [128, T], F32, kind="ExternalInput")
    A0 = nc.dram_tensor("A0", [128, 128], F32, kind="ExternalInput")
    ones_c = nc.dram_tensor("ones_c", [128, 128], F32R, kind="ExternalInput")
    outp = nc.dram_tensor("outp", [T, C], F32, kind="ExternalOutput")

    with tile.TileContext(nc) as tc:
        for _ in range(reps):
            with ExitStack() as ctx:
                _body(nc, tc,
                      (xT[:], wqk[:], wv[:], wp[:], dec[:], Lc[:], A0[:],
                       ones_c[:], outp[:]), ctx)
    nc.compile()
    _CACHE[key] = nc
    return nc


def _in_maps(x, w_attn, w_proj, decay_raw):
    x = np.asarray(x, dtype=np.float32)
    w_attn = np.asarray(w_attn, dtype=np.float32)
    w_proj = np.asarray(w_proj, dtype=np.float32)
    decay_raw = np.asarray(decay_raw, dtype=np.float32)

    d = np.arange(T)[None, :] - np.arange(128)[:, None]
    Lc = np.log1p(np.maximum(d, 0)).astype(np.float32)
    A0 = (np.arange(128)[None, :] >= np.arange(128)[:, None]).astype(np.float32)
    ones_c = np.ones((128, 128), dtype=np.float32)

    maps = []
    for c in range(N_CORES):
        b, g = c // 2, c % 2
        q0 = g * (HG * D)
        maps.append({
            "xT": np.ascontiguousarray(x[b].T),
            "wqk": np.ascontiguousarray(
                np.concatenate(
                    [w_attn[:, q0 : q0 + HG * D] * np.float32(0.125),
                     w_attn[:, C + q0 : C + q0 + HG * D]], axis=1)),
            "wv": np.ascontiguousarray(w_attn[:, 2 * C + q0 : 2 * C + q0 + HG * D]),
            "wp": np.ascontiguousarray(w_proj[q0 : q0 + HG * D, :]),
            "dec": np.ascontiguousarray(decay_raw[HG * g : HG * (g + 1)][None, :]),
            "Lc": Lc,
            "A0": A0,
            "ones_c": ones_c,
        })
    return maps


def kernel(x, w_attn, w_proj, decay_raw):
    nc = _build()
    maps = _in_maps(x, w_attn, w_proj, decay_raw)
    res = run_bass_kernel_spmd(nc, maps, list(range(N_CORES)))
    out = np.stack(
        [res.results[2 * b]["outp"] + res.results[2 * b + 1]["outp"]
         for b in range(B)]
    ).astype(np.float32)
    return out


def bench(inputs, iters=20, reps=1):
    """Time repeated on-device executions (inputs pre-placed, async dispatch).

    Returns estimated per-execution time in ns. Not used by the grading
    entry point; test.py calls this for the HW time estimate.
    """
    import time
    import jax
    from jax.experimental.shard_map import shard_map
    from jax.sharding import Mesh, NamedSharding, PartitionSpec
    from concourse import bass2jax

    nc = _build(reps)
    maps = _in_maps(inputs["x"], inputs["w_attn"], inputs["w_proj"],
                    inputs["decay_raw"])
    bass2jax.install_neuronx_cc_hook()

    in_specs_list = []   # (name, shape, np dtype)
    out_names, out_avals = [], []
    for alloc in nc.m.functions[0].allocations:
        if not isinstance(alloc, mybir.MemoryLocationSet):
            continue
        name = alloc.memorylocations[0].name
        if alloc.kind == "ExternalInput":
            in_specs_list.append(
                (name, tuple(alloc.tensor_shape), mybir.dt.np(alloc.dtype)))
        elif alloc.kind == "ExternalOutput":
            out_names.append(name)
            shape = tuple(alloc.tensor_shape)
            dtype = mybir.dt.np(alloc.dtype)
            out_avals.append(jax.core.ShapedArray(shape, dtype))
    in_names = [n for (n, _, _) in in_specs_list]
    all_names = tuple(in_names + out_names)

    def _b(*args):
        outs = bass2jax._bass_exec_p.bind(
            *args, out_avals=tuple(out_avals), in_names=all_names,
            out_names=tuple(out_names), lowering_input_output_aliases=(),
            sim_require_finite=True, sim_require_nnan=True, nc=nc)
        return tuple(outs)

    devices = jax.devices()[:N_CORES]
    mesh = Mesh(np.asarray(devices), ("core",))
    nin = len(in_specs_list) + len(out_names)
    fn = jax.jit(shard_map(
        _b, mesh=mesh,
        in_specs=(PartitionSpec("core"),) * nin,
        out_specs=(PartitionSpec("core"),) * len(out_names),
        check_rep=False))

    concat = []
    for (name, shape, dtype) in in_specs_list:
        percore = [
            np.asarray(maps[c][name]) if name in maps[c]
            else np.zeros(shape, dtype)
            for c in range(N_CORES)
        ]
        concat.append(np.concatenate(percore, axis=0))
    for av in out_avals:
        concat.append(
            np.zeros((N_CORES * av.shape[0], *av.shape[1:]), av.dtype))
    sharding = NamedSharding(mesh, PartitionSpec("core"))
    dev_args = [jax.device_put(a, sharding) for a in concat]

    out = fn(*dev_args)
    jax.block_until_ready(out)
    t0 = time.perf_counter()
    for _ in range(iters):
        out = fn(*dev_args)
    jax.block_until_ready(out)
    t1 = time.perf_counter()
    return (t1 - t0) / iters * 1e9
